# revision 31
# baseline (speedup 1.0000x reference)
"""Trainium2 Bass kernel for nn_NeuronGraph_43336220017086.

Reference semantics:
    h_prev = concat(obs, current[N_IN:])            # [N]
    pre    = W @ h_prev + bias                      # [N]
    pre[rec_dst] += rec_w * history[rec_src, rec_delay]
    return tanh(pre)[-N_OUT:]                       # [N_OUT]

Only the last N_OUT=32 rows of `pre` reach the output, so the kernel
computes exactly those rows.  Work is sharded across 8 NeuronCores:
core c owns output rows [N-32+4c, N-32+4c+4).

Per core everything is folded into ONE fused DVE multiply-accumulate over
partition p = r*32 + q (r = local output row, q = 0..31):
  - dense part: in0[p, 0:256]   = W[r0+r, q*256:(q+1)*256]
                in1[p, 0:256]   = h_prev[q*256:(q+1)*256]
  - sparse part (recurrent edges, paged): each referenced 256-wide chunk
    of history.reshape(-1) is shipped once per (row, slot q):
                in1[p, 256:512] = hist_chunk(chunk_id[r, q])
                in0[p, 256:512] = per-element summed rec_w at the edge
                                  offsets inside that chunk (else 0)
  One scalar_tensor_tensor with accum_out produces 128 partial sums; a
  single tensor-engine matmul with the 0/1 matrix R[p,r]=(p//32==r)
  contracts them; ACT applies tanh(pre + bias); 4 results DMA out.

The inputs ride two DMAs on separate HWDGE queues into one SBUF tile;
block-strided access patterns present {W|Mw} and {h|histch} as single
operands.  A post-compile surgery pass hides the semaphore reset and one
all-engine barrier under the NEFF engine preamble and hoists the DMAs to
the entry block.

If an output row's edges ever reference more than 32 distinct chunks
(not observed; ~impossible for this distribution), the kernel falls back
to an indirect-DMA gather variant.
"""

import sys

for _p in ("/opt/trn_rl_repo", "/root/.axon_site/_ro/trn_rl_repo"):
    if _p not in sys.path:
        sys.path.insert(0, _p)

import numpy as np

import concourse.bacc as bacc
import concourse.bass as bass
import concourse.mybir as mybir
from concourse.bass_utils import run_bass_kernel_spmd
from concourse.tile import TileContext

N = 8192
N_IN = 64
N_OUT = 32
D = 4
N_CORES = 8
R_PER_CORE = N_OUT // N_CORES      # 4 output rows per core
KC = 128 // R_PER_CORE             # 32 partition groups per row
CHUNK = N // KC                    # 256 dense columns per partition
CH = 256                           # history page size (elements)
SLOTS = KC                         # history pages per output row
NSMALL = R_PER_CORE + 1            # rmat(4) + bias(1)
TCOLS = 4 * CH + NSMALL            # W | h | Mw | histch | small
CAP_E = 128                        # fallback gather capacity

_F32 = mybir.dt.float32
_I32 = mybir.dt.int32

_NC = {}


def _build_nc_fused():
    nc = bacc.Bacc(
        "TRN2", target_bir_lowering=False, debug=False, num_devices=N_CORES
    )
    packa = nc.dram_tensor(
        "packa", [128, 2 * CH + NSMALL], _F32, kind="ExternalInput"
    )
    packb = nc.dram_tensor("packb", [128, 2 * CH], _F32, kind="ExternalInput")
    out = nc.dram_tensor("out", [R_PER_CORE, 1], _F32, kind="ExternalOutput")

    with TileContext(nc) as tc:
        with (
            tc.tile_pool(name="sbuf", bufs=1) as pool,
            tc.tile_pool(name="psum", bufs=1, space="PSUM") as pp,
        ):
            A = 2 * CH + NSMALL
            t = pool.tile([128, TCOLS], _F32)
            # same HWDGE queue: FIFO guarantees pack A completes (and its
            # completion semaphore fully trickles) before pack B's, so the
            # dense half runs during pack B's completion.
            nc.sync.dma_start(out=t[:, 0:A], in_=packa[:, :])
            nc.sync.dma_start(out=t[:, A:TCOLS], in_=packb[:, :])

            r_t = t[:, 2 * CH : 2 * CH + R_PER_CORE]
            b_t = t[0:R_PER_CORE, 2 * CH + R_PER_CORE : 2 * CH + R_PER_CORE + 1]

            prod_d = pool.tile([128, CH], _F32)
            acc_d = pool.tile([128, 1], _F32)
            nc.vector.scalar_tensor_tensor(
                out=prod_d[:, :],
                in0=t[:, 0:CH],
                scalar=1.0,
                in1=t[:, CH : 2 * CH],
                op0=mybir.AluOpType.mult,
                op1=mybir.AluOpType.mult,
                accum_out=acc_d[:, :],
            )
            prod_s = pool.tile([128, CH], _F32)
            acc_s = pool.tile([128, 1], _F32)
            nc.vector.scalar_tensor_tensor(
                out=prod_s[:, :],
                in0=t[:, A : A + CH],
                scalar=1.0,
                in1=t[:, A + CH : A + 2 * CH],
                op0=mybir.AluOpType.mult,
                op1=mybir.AluOpType.mult,
                accum_out=acc_s[:, :],
            )

            ps = pp.tile([R_PER_CORE, 1], _F32)
            nc.tensor.matmul(
                out=ps[:, :], lhsT=r_t, rhs=acc_d[:, :], start=True, stop=False
            )
            nc.tensor.matmul(
                out=ps[:, :], lhsT=r_t, rhs=acc_s[:, :], start=False, stop=True
            )

            o_t = pool.tile([R_PER_CORE, 1], _F32)
            nc.scalar.activation(
                o_t[:, :],
                ps[:, :],
                mybir.ActivationFunctionType.Tanh,
                bias=b_t,
                scale=1.0,
            )
            nc.sync.dma_start(out=out[:, :], in_=o_t[:, :], single_packet=True)

    nc.compile()
    _surgery(nc)
    return nc


def _build_nc_gather(cap=CAP_E):
    nc = bacc.Bacc(
        "TRN2", target_bir_lowering=False, debug=False, num_devices=N_CORES
    )
    nsm = 2 * R_PER_CORE + 2
    small = nc.dram_tensor("small", [128, nsm], _F32, kind="ExternalInput")
    big = nc.dram_tensor("big", [128, 2 * CHUNK], _F32, kind="ExternalInput")
    hist = nc.dram_tensor("hist", [N * D, 1], _F32, kind="ExternalInput")
    out = nc.dram_tensor("out", [R_PER_CORE, 1], _F32, kind="ExternalOutput")

    with TileContext(nc) as tc:
        with (
            tc.tile_pool(name="sbuf", bufs=1) as pool,
            tc.tile_pool(name="psum", bufs=1, space="PSUM") as pp,
        ):
            sm_t = pool.tile([128, nsm], _F32)
            nc.sync.dma_start(out=sm_t[:, :], in_=small[:, :], single_packet=True)
            r_t = sm_t[:, 0:R_PER_CORE]
            s_t = sm_t[0:cap, R_PER_CORE : 2 * R_PER_CORE]
            e_t = sm_t[0:cap, 2 * R_PER_CORE : 2 * R_PER_CORE + 1].bitcast(_I32)
            b_t = sm_t[0:R_PER_CORE, 2 * R_PER_CORE + 1 : 2 * R_PER_CORE + 2]

            g_t = pool.tile([cap, 1], _F32)
            nc.gpsimd.indirect_dma_start(
                out=g_t[:, :],
                out_offset=None,
                in_=hist[:, :],
                in_offset=bass.IndirectOffsetOnAxis(ap=e_t, axis=0),
            )

            w_t = pool.tile([128, 2 * CHUNK], _F32)
            nc.scalar.dma_start(out=w_t[:, :], in_=big[:, :])

            prod = pool.tile([128, CHUNK], _F32)
            part = pool.tile([128, 1], _F32)
            nc.vector.scalar_tensor_tensor(
                out=prod[:, :],
                in0=w_t[:, 0:CHUNK],
                scalar=1.0,
                in1=w_t[:, CHUNK : 2 * CHUNK],
                op0=mybir.AluOpType.mult,
                op1=mybir.AluOpType.mult,
                accum_out=part[:, :],
            )

            ps = pp.tile([R_PER_CORE, 1], _F32)
            nc.tensor.matmul(
                out=ps[:, :], lhsT=r_t, rhs=part[:, :], start=True, stop=False
            )
            nc.tensor.matmul(
                out=ps[:, :], lhsT=s_t, rhs=g_t[:, :], start=False, stop=True
            )

            o_t = pool.tile([R_PER_CORE, 1], _F32)
            nc.scalar.activation(
                o_t[:, :],
                ps[:, :],
                mybir.ActivationFunctionType.Tanh,
                bias=b_t,
                scale=1.0,
            )
            nc.sync.dma_start(out=out[:, :], in_=o_t[:, :], single_packet=True)

    nc.compile()
    _surgery(nc)
    return nc


def _surgery(nc):
    """Post-compile BIR surgery to shave fixed overhead:
    - drop the entry-block const memsets + all-engine barrier,
    - move the semaphore reset + range-clear + one all-engine barrier from
      the tail into the entry block (hidden under the NEFF preamble),
    - hoist the input DMAs into the entry block right after that barrier,
    - drop the second tail barrier; keep only the SP flush waits + drain,
    - merge the three basic blocks into one linear stream.
    """
    blocks = nc.m.functions[0].blocks
    if len(blocks) != 3:
        return
    main_bb, tile_bb, end_bb = blocks[0], blocks[1], blocks[2]

    hoist, rest = [], []
    for i in tile_bb.instructions:
        if isinstance(i, mybir.InstDMACopy) and getattr(i.ins[0], "memref", None) in (
            "small",
            "big",
            "hist",
            "packa",
            "packb",
        ):
            if getattr(i.ins[0], "memref", None) == "hist":
                i.single_packet = True  # one-engine completion for the gather
            hoist.append(i)
        else:
            rest.append(i)

    keep_head = [i for i in main_bb.instructions if isinstance(i, mybir.InstCall)]
    main_il = main_bb.instructions
    tile_il = tile_bb.instructions

    end_il = end_bb.instructions
    keep_end, entry_sync = [], []
    seen_isa = False
    for i in end_il:
        if isinstance(i, mybir.InstISA):
            entry_sync.append(i)
            seen_isa = True
        elif isinstance(i, mybir.InstDrain):
            if (
                i.engine == mybir.EngineType.SP
                and i.has_wait()
                and "barrier_" not in i.concise()
            ):
                keep_end.append(i)  # SP flush drain
            elif seen_isa:
                pass  # second-barrier drains: drop
            else:
                entry_sync.append(i)
        elif isinstance(i, mybir.InstEventSemaphore):
            if i.engine == mybir.EngineType.SP and not i.name.startswith("barrier"):
                keep_end.append(i)  # global-clock DMA waits
            elif seen_isa:
                pass  # second barrier: drop
            else:
                entry_sync.append(i)
        else:
            keep_end.append(i)

    pool_reset = [
        i
        for i in entry_sync
        if isinstance(i, mybir.InstISA)
        or (isinstance(i, mybir.InstDrain) and "is_reset_sema=True" in i.concise())
    ]
    barrier_a = [i for i in entry_sync if i not in pool_reset]

    body = [i for i in rest if not isinstance(i, mybir.InstUnconditionalBranch)]
    # Input DMAs issue before the entry barrier: their semaphore increments
    # trail the Pool range-clear by construction (issue+transfer+receipt far
    # exceeds Pool's two-instruction path to the clear), and the barrier's
    # engine drains then overlap the transfers.
    main_il[:] = keep_head + pool_reset + hoist + barrier_a + body + keep_end
    tile_il[:] = []
    end_il[:] = []
    del nc.m.functions[0].blocks[1:]


def _get_nc(mode):
    if mode not in _NC:
        if mode == "fused":
            _NC[mode] = _build_nc_fused()
        else:
            _NC[mode] = _build_nc_gather(CAP_E)
    return _NC[mode]


def _prep_in_maps(obs, W, bias, current, history, rec_w, rec_src, rec_dst, rec_delay):
    obs = np.asarray(obs, np.float32)
    W = np.asarray(W, np.float32)
    bias = np.asarray(bias, np.float32)
    current = np.asarray(current, np.float32)
    history = np.ascontiguousarray(np.asarray(history, np.float32))
    rec_w = np.asarray(rec_w, np.float32)
    rec_src = np.asarray(rec_src).astype(np.int64)
    rec_dst = np.asarray(rec_dst).astype(np.int64)
    rec_delay = np.asarray(rec_delay).astype(np.int64)

    h = np.concatenate([obs, current[N_IN:]]).astype(np.float32)
    histf = history.reshape(N * D)
    hrep = np.tile(h.reshape(KC, CHUNK), (R_PER_CORE, 1))
    flat_all = rec_src * D + rec_delay

    # ---- fused (paged-history) layout ----
    in_maps = []
    fits = True
    for core in range(N_CORES):
        r0 = N - N_OUT + R_PER_CORE * core
        packa = np.zeros((128, 2 * CH + NSMALL), np.float32)
        packa[:, 0:CH] = (
            W[r0 : r0 + R_PER_CORE].reshape(R_PER_CORE, KC, CHUNK).reshape(128, CHUNK)
        )
        packa[:, CH : 2 * CH] = hrep
        packa[np.arange(128), 2 * CH + np.arange(128) // KC] = 1.0
        packa[0:R_PER_CORE, 2 * CH + R_PER_CORE] = bias[r0 : r0 + R_PER_CORE]

        packb = np.zeros((128, 2 * CH), np.float32)
        for r in range(R_PER_CORE):
            sel = rec_dst == r0 + r
            fl = flat_all[sel]
            ws = rec_w[sel]
            chunks = fl // CH
            offs = fl % CH
            uc = np.unique(chunks)
            if uc.shape[0] > SLOTS:
                fits = False
                break
            slot = {int(c): j for j, c in enumerate(uc)}
            for j, c in enumerate(uc):
                packb[r * KC + j, CH : 2 * CH] = histf[c * CH : (c + 1) * CH]
            prows = np.array([r * KC + slot[int(c)] for c in chunks], np.int64)
            np.add.at(packb, (prows, offs), ws)
        if not fits:
            break
        in_maps.append({"packa": packa, "packb": packb})

    if fits:
        return in_maps, "fused"

    # ---- fallback: indirect-DMA gather layout ----
    histc = histf.reshape(N * D, 1)
    in_maps = []
    for core in range(N_CORES):
        r0 = N - N_OUT + R_PER_CORE * core
        big = np.empty((128, 2 * CHUNK), np.float32)
        big[:, 0:CHUNK] = (
            W[r0 : r0 + R_PER_CORE].reshape(R_PER_CORE, KC, CHUNK).reshape(128, CHUNK)
        )
        big[:, CHUNK:] = hrep

        sel = (rec_dst >= r0) & (rec_dst < r0 + R_PER_CORE)
        ew = rec_w[sel]
        flat = flat_all[sel]
        edloc = rec_dst[sel] - r0
        ne = int(flat.shape[0])
        if ne > CAP_E:
            key = flat * N_OUT + edloc
            uk, inv = np.unique(key, return_inverse=True)
            acc = np.zeros(uk.shape[0], np.float32)
            np.add.at(acc, inv, ew)
            flat = (uk // N_OUT).astype(np.int64)
            edloc = (uk % N_OUT).astype(np.int64)
            ew = acc
            ne = uk.shape[0]
            assert ne <= CAP_E, f"edge capacity exceeded: {ne}"

        nsm = 2 * R_PER_CORE + 2
        small = np.zeros((128, nsm), np.float32)
        small[np.arange(128), np.arange(128) // KC] = 1.0
        small[np.arange(ne), R_PER_CORE + edloc] = ew
        eidx = np.zeros((CAP_E,), np.int32)
        eidx[:ne] = flat.astype(np.int32)
        small[0:CAP_E, 2 * R_PER_CORE] = eidx.view(np.float32)
        small[0:R_PER_CORE, 2 * R_PER_CORE + 1] = bias[r0 : r0 + R_PER_CORE]
        in_maps.append({"small": small, "big": big, "hist": histc})
    return in_maps, "gather"


def _run(in_maps, mode="fused", trace=False, **kw):
    nc = _get_nc(mode)
    res = run_bass_kernel_spmd(
        nc, in_maps, core_ids=list(range(N_CORES)), trace=trace, **kw
    )
    outs = [res.results[c]["out"].reshape(R_PER_CORE) for c in range(N_CORES)]
    return np.concatenate(outs).astype(np.float32), res


def kernel(**inputs):
    in_maps, mode = _prep_in_maps(**inputs)
    out, _ = _run(in_maps, mode, trace=False)
    return out


# revision 32
# speedup vs baseline: 1.0296x; 1.0296x over previous
"""Trainium2 Bass kernel for nn_NeuronGraph_43336220017086.

Reference semantics:
    h_prev = concat(obs, current[N_IN:])            # [N]
    pre    = W @ h_prev + bias                      # [N]
    pre[rec_dst] += rec_w * history[rec_src, rec_delay]
    return tanh(pre)[-N_OUT:]                       # [N_OUT]

Only the last N_OUT=32 rows of `pre` reach the output, so the kernel
computes exactly those rows.  Work is sharded across 8 NeuronCores:
core c owns output rows [N-32+4c, N-32+4c+4).

Per core everything is folded into ONE fused DVE multiply-accumulate over
partition p = r*32 + q (r = local output row, q = 0..31):
  - dense part: in0[p, 0:256]   = W[r0+r, q*256:(q+1)*256]
                in1[p, 0:256]   = h_prev[q*256:(q+1)*256]
  - sparse part (recurrent edges, paged): each referenced 256-wide chunk
    of history.reshape(-1) is shipped once per (row, slot q):
                in1[p, 256:512] = hist_chunk(chunk_id[r, q])
                in0[p, 256:512] = per-element summed rec_w at the edge
                                  offsets inside that chunk (else 0)
  One scalar_tensor_tensor with accum_out produces 128 partial sums; a
  single tensor-engine matmul with the 0/1 matrix R[p,r]=(p//32==r)
  contracts them; ACT applies tanh(pre + bias); 4 results DMA out.

The inputs ride two DMAs on separate HWDGE queues into one SBUF tile;
block-strided access patterns present {W|Mw} and {h|histch} as single
operands.  A post-compile surgery pass hides the semaphore reset and one
all-engine barrier under the NEFF engine preamble and hoists the DMAs to
the entry block.

If an output row's edges ever reference more than 32 distinct chunks
(not observed; ~impossible for this distribution), the kernel falls back
to an indirect-DMA gather variant.
"""

import sys

for _p in ("/opt/trn_rl_repo", "/root/.axon_site/_ro/trn_rl_repo"):
    if _p not in sys.path:
        sys.path.insert(0, _p)

import numpy as np

import concourse.bacc as bacc
import concourse.bass as bass
import concourse.mybir as mybir
from concourse.bass_utils import run_bass_kernel_spmd
from concourse.tile import TileContext

N = 8192
N_IN = 64
N_OUT = 32
D = 4
N_CORES = 8
R_PER_CORE = N_OUT // N_CORES      # 4 output rows per core
KC = 128 // R_PER_CORE             # 32 partition groups per row
CHUNK = N // KC                    # 256 dense columns per partition
CH = 256                           # history page size (elements)
SLOTS = KC                         # history pages per output row
NSMALL = R_PER_CORE + 1            # rmat(4) + bias(1)
TCOLS = 4 * CH + NSMALL            # W | h | Mw | histch | small
CAP_E = 128                        # fallback gather capacity

_F32 = mybir.dt.float32
_I32 = mybir.dt.int32

_NC = {}


def _build_nc_fused():
    nc = bacc.Bacc(
        "TRN2", target_bir_lowering=False, debug=False, num_devices=N_CORES
    )
    packa = nc.dram_tensor(
        "packa", [128, 2 * CH + NSMALL], _F32, kind="ExternalInput"
    )
    packb = nc.dram_tensor("packb", [128, 2 * CH], _F32, kind="ExternalInput")
    out = nc.dram_tensor("out", [R_PER_CORE, 1], _F32, kind="ExternalOutput")

    with TileContext(nc) as tc:
        with (
            tc.tile_pool(name="sbuf", bufs=1) as pool,
            tc.tile_pool(name="psum", bufs=1, space="PSUM") as pp,
        ):
            A = 2 * CH + NSMALL
            t = pool.tile([128, TCOLS], _F32)
            # separate HWDGE queues: transfers run in parallel; the dense
            # half consumes pack A while pack B's completion trickles in.
            nc.sync.dma_start(out=t[:, 0:A], in_=packa[:, :])
            nc.scalar.dma_start(out=t[:, A:TCOLS], in_=packb[:, :])

            r_t = t[:, 2 * CH : 2 * CH + R_PER_CORE]
            b_t = t[0:R_PER_CORE, 2 * CH + R_PER_CORE : 2 * CH + R_PER_CORE + 1]

            prod_d = pool.tile([128, CH], _F32)
            acc_d = pool.tile([128, 1], _F32)
            nc.vector.scalar_tensor_tensor(
                out=prod_d[:, :],
                in0=t[:, 0:CH],
                scalar=1.0,
                in1=t[:, CH : 2 * CH],
                op0=mybir.AluOpType.mult,
                op1=mybir.AluOpType.mult,
                accum_out=acc_d[:, :],
            )
            prod_s = pool.tile([128, CH], _F32)
            acc_s = pool.tile([128, 1], _F32)
            nc.vector.scalar_tensor_tensor(
                out=prod_s[:, :],
                in0=t[:, A : A + CH],
                scalar=1.0,
                in1=t[:, A + CH : A + 2 * CH],
                op0=mybir.AluOpType.mult,
                op1=mybir.AluOpType.mult,
                accum_out=acc_s[:, :],
            )

            ps = pp.tile([R_PER_CORE, 1], _F32)
            nc.tensor.matmul(
                out=ps[:, :], lhsT=r_t, rhs=acc_d[:, :], start=True, stop=False
            )
            nc.tensor.matmul(
                out=ps[:, :], lhsT=r_t, rhs=acc_s[:, :], start=False, stop=True
            )

            o_t = pool.tile([R_PER_CORE, 1], _F32)
            nc.scalar.activation(
                o_t[:, :],
                ps[:, :],
                mybir.ActivationFunctionType.Tanh,
                bias=b_t,
                scale=1.0,
            )
            nc.sync.dma_start(out=out[:, :], in_=o_t[:, :], single_packet=True)

    nc.compile()
    _surgery(nc)
    return nc


def _build_nc_gather(cap=CAP_E):
    nc = bacc.Bacc(
        "TRN2", target_bir_lowering=False, debug=False, num_devices=N_CORES
    )
    nsm = 2 * R_PER_CORE + 2
    small = nc.dram_tensor("small", [128, nsm], _F32, kind="ExternalInput")
    big = nc.dram_tensor("big", [128, 2 * CHUNK], _F32, kind="ExternalInput")
    hist = nc.dram_tensor("hist", [N * D, 1], _F32, kind="ExternalInput")
    out = nc.dram_tensor("out", [R_PER_CORE, 1], _F32, kind="ExternalOutput")

    with TileContext(nc) as tc:
        with (
            tc.tile_pool(name="sbuf", bufs=1) as pool,
            tc.tile_pool(name="psum", bufs=1, space="PSUM") as pp,
        ):
            sm_t = pool.tile([128, nsm], _F32)
            nc.sync.dma_start(out=sm_t[:, :], in_=small[:, :], single_packet=True)
            r_t = sm_t[:, 0:R_PER_CORE]
            s_t = sm_t[0:cap, R_PER_CORE : 2 * R_PER_CORE]
            e_t = sm_t[0:cap, 2 * R_PER_CORE : 2 * R_PER_CORE + 1].bitcast(_I32)
            b_t = sm_t[0:R_PER_CORE, 2 * R_PER_CORE + 1 : 2 * R_PER_CORE + 2]

            g_t = pool.tile([cap, 1], _F32)
            nc.gpsimd.indirect_dma_start(
                out=g_t[:, :],
                out_offset=None,
                in_=hist[:, :],
                in_offset=bass.IndirectOffsetOnAxis(ap=e_t, axis=0),
            )

            w_t = pool.tile([128, 2 * CHUNK], _F32)
            nc.scalar.dma_start(out=w_t[:, :], in_=big[:, :])

            prod = pool.tile([128, CHUNK], _F32)
            part = pool.tile([128, 1], _F32)
            nc.vector.scalar_tensor_tensor(
                out=prod[:, :],
                in0=w_t[:, 0:CHUNK],
                scalar=1.0,
                in1=w_t[:, CHUNK : 2 * CHUNK],
                op0=mybir.AluOpType.mult,
                op1=mybir.AluOpType.mult,
                accum_out=part[:, :],
            )

            ps = pp.tile([R_PER_CORE, 1], _F32)
            nc.tensor.matmul(
                out=ps[:, :], lhsT=r_t, rhs=part[:, :], start=True, stop=False
            )
            nc.tensor.matmul(
                out=ps[:, :], lhsT=s_t, rhs=g_t[:, :], start=False, stop=True
            )

            o_t = pool.tile([R_PER_CORE, 1], _F32)
            nc.scalar.activation(
                o_t[:, :],
                ps[:, :],
                mybir.ActivationFunctionType.Tanh,
                bias=b_t,
                scale=1.0,
            )
            nc.sync.dma_start(out=out[:, :], in_=o_t[:, :], single_packet=True)

    nc.compile()
    _surgery(nc)
    return nc


def _surgery(nc):
    """Post-compile BIR surgery to shave fixed overhead:
    - drop the entry-block const memsets + all-engine barrier,
    - move the semaphore reset + range-clear + one all-engine barrier from
      the tail into the entry block (hidden under the NEFF preamble),
    - hoist the input DMAs into the entry block right after that barrier,
    - drop the second tail barrier; keep only the SP flush waits + drain,
    - merge the three basic blocks into one linear stream.
    """
    blocks = nc.m.functions[0].blocks
    if len(blocks) != 3:
        return
    main_bb, tile_bb, end_bb = blocks[0], blocks[1], blocks[2]

    hoist, rest = [], []
    for i in tile_bb.instructions:
        if isinstance(i, mybir.InstDMACopy) and getattr(i.ins[0], "memref", None) in (
            "small",
            "big",
            "hist",
            "packa",
            "packb",
        ):
            if getattr(i.ins[0], "memref", None) == "hist":
                i.single_packet = True  # one-engine completion for the gather
            hoist.append(i)
        else:
            rest.append(i)

    keep_head = [i for i in main_bb.instructions if isinstance(i, mybir.InstCall)]
    main_il = main_bb.instructions
    tile_il = tile_bb.instructions

    end_il = end_bb.instructions
    keep_end, entry_sync = [], []
    seen_isa = False
    for i in end_il:
        if isinstance(i, mybir.InstISA):
            entry_sync.append(i)
            seen_isa = True
        elif isinstance(i, mybir.InstDrain):
            if (
                i.engine == mybir.EngineType.SP
                and i.has_wait()
                and "barrier_" not in i.concise()
            ):
                keep_end.append(i)  # SP flush drain
            elif seen_isa:
                pass  # second-barrier drains: drop
            else:
                entry_sync.append(i)
        elif isinstance(i, mybir.InstEventSemaphore):
            if i.engine == mybir.EngineType.SP and not i.name.startswith("barrier"):
                keep_end.append(i)  # global-clock DMA waits
            elif seen_isa:
                pass  # second barrier: drop
            else:
                entry_sync.append(i)
        else:
            keep_end.append(i)

    pool_reset = [
        i
        for i in entry_sync
        if isinstance(i, mybir.InstISA)
        or (isinstance(i, mybir.InstDrain) and "is_reset_sema=True" in i.concise())
    ]
    barrier_a = [i for i in entry_sync if i not in pool_reset]

    body = [i for i in rest if not isinstance(i, mybir.InstUnconditionalBranch)]
    # Input DMAs issue before the entry barrier: their semaphore increments
    # trail the Pool range-clear by construction (issue+transfer+receipt far
    # exceeds Pool's two-instruction path to the clear), and the barrier's
    # engine drains then overlap the transfers.
    main_il[:] = keep_head + pool_reset + hoist + barrier_a + body + keep_end
    tile_il[:] = []
    end_il[:] = []
    del nc.m.functions[0].blocks[1:]


def _get_nc(mode):
    if mode not in _NC:
        if mode == "fused":
            _NC[mode] = _build_nc_fused()
        else:
            _NC[mode] = _build_nc_gather(CAP_E)
    return _NC[mode]


def _prep_in_maps(obs, W, bias, current, history, rec_w, rec_src, rec_dst, rec_delay):
    obs = np.asarray(obs, np.float32)
    W = np.asarray(W, np.float32)
    bias = np.asarray(bias, np.float32)
    current = np.asarray(current, np.float32)
    history = np.ascontiguousarray(np.asarray(history, np.float32))
    rec_w = np.asarray(rec_w, np.float32)
    rec_src = np.asarray(rec_src).astype(np.int64)
    rec_dst = np.asarray(rec_dst).astype(np.int64)
    rec_delay = np.asarray(rec_delay).astype(np.int64)

    h = np.concatenate([obs, current[N_IN:]]).astype(np.float32)
    histf = history.reshape(N * D)
    hrep = np.tile(h.reshape(KC, CHUNK), (R_PER_CORE, 1))
    flat_all = rec_src * D + rec_delay

    # ---- fused (paged-history) layout ----
    in_maps = []
    fits = True
    for core in range(N_CORES):
        r0 = N - N_OUT + R_PER_CORE * core
        packa = np.zeros((128, 2 * CH + NSMALL), np.float32)
        packa[:, 0:CH] = (
            W[r0 : r0 + R_PER_CORE].reshape(R_PER_CORE, KC, CHUNK).reshape(128, CHUNK)
        )
        packa[:, CH : 2 * CH] = hrep
        packa[np.arange(128), 2 * CH + np.arange(128) // KC] = 1.0
        packa[0:R_PER_CORE, 2 * CH + R_PER_CORE] = bias[r0 : r0 + R_PER_CORE]

        packb = np.zeros((128, 2 * CH), np.float32)
        for r in range(R_PER_CORE):
            sel = rec_dst == r0 + r
            fl = flat_all[sel]
            ws = rec_w[sel]
            chunks = fl // CH
            offs = fl % CH
            uc = np.unique(chunks)
            if uc.shape[0] > SLOTS:
                fits = False
                break
            slot = {int(c): j for j, c in enumerate(uc)}
            for j, c in enumerate(uc):
                packb[r * KC + j, CH : 2 * CH] = histf[c * CH : (c + 1) * CH]
            prows = np.array([r * KC + slot[int(c)] for c in chunks], np.int64)
            np.add.at(packb, (prows, offs), ws)
        if not fits:
            break
        in_maps.append({"packa": packa, "packb": packb})

    if fits:
        return in_maps, "fused"

    # ---- fallback: indirect-DMA gather layout ----
    histc = histf.reshape(N * D, 1)
    in_maps = []
    for core in range(N_CORES):
        r0 = N - N_OUT + R_PER_CORE * core
        big = np.empty((128, 2 * CHUNK), np.float32)
        big[:, 0:CHUNK] = (
            W[r0 : r0 + R_PER_CORE].reshape(R_PER_CORE, KC, CHUNK).reshape(128, CHUNK)
        )
        big[:, CHUNK:] = hrep

        sel = (rec_dst >= r0) & (rec_dst < r0 + R_PER_CORE)
        ew = rec_w[sel]
        flat = flat_all[sel]
        edloc = rec_dst[sel] - r0
        ne = int(flat.shape[0])
        if ne > CAP_E:
            key = flat * N_OUT + edloc
            uk, inv = np.unique(key, return_inverse=True)
            acc = np.zeros(uk.shape[0], np.float32)
            np.add.at(acc, inv, ew)
            flat = (uk // N_OUT).astype(np.int64)
            edloc = (uk % N_OUT).astype(np.int64)
            ew = acc
            ne = uk.shape[0]
            assert ne <= CAP_E, f"edge capacity exceeded: {ne}"

        nsm = 2 * R_PER_CORE + 2
        small = np.zeros((128, nsm), np.float32)
        small[np.arange(128), np.arange(128) // KC] = 1.0
        small[np.arange(ne), R_PER_CORE + edloc] = ew
        eidx = np.zeros((CAP_E,), np.int32)
        eidx[:ne] = flat.astype(np.int32)
        small[0:CAP_E, 2 * R_PER_CORE] = eidx.view(np.float32)
        small[0:R_PER_CORE, 2 * R_PER_CORE + 1] = bias[r0 : r0 + R_PER_CORE]
        in_maps.append({"small": small, "big": big, "hist": histc})
    return in_maps, "gather"


def _run(in_maps, mode="fused", trace=False, **kw):
    nc = _get_nc(mode)
    res = run_bass_kernel_spmd(
        nc, in_maps, core_ids=list(range(N_CORES)), trace=trace, **kw
    )
    outs = [res.results[c]["out"].reshape(R_PER_CORE) for c in range(N_CORES)]
    return np.concatenate(outs).astype(np.float32), res


def kernel(**inputs):
    in_maps, mode = _prep_in_maps(**inputs)
    out, _ = _run(in_maps, mode, trace=False)
    return out


# revision 33
# speedup vs baseline: 1.0436x; 1.0136x over previous
"""Trainium2 Bass kernel for nn_NeuronGraph_43336220017086.

Reference semantics:
    h_prev = concat(obs, current[N_IN:])            # [N]
    pre    = W @ h_prev + bias                      # [N]
    pre[rec_dst] += rec_w * history[rec_src, rec_delay]
    return tanh(pre)[-N_OUT:]                       # [N_OUT]

Only the last N_OUT=32 rows of `pre` reach the output, so the kernel
computes exactly those rows.  Work is sharded across 8 NeuronCores:
core c owns output rows [N-32+4c, N-32+4c+4).

Per core everything is folded into ONE fused DVE multiply-accumulate over
partition p = r*32 + q (r = local output row, q = 0..31):
  - dense part: in0[p, 0:256]   = W[r0+r, q*256:(q+1)*256]
                in1[p, 0:256]   = h_prev[q*256:(q+1)*256]
  - sparse part (recurrent edges, paged): each referenced 256-wide chunk
    of history.reshape(-1) is shipped once per (row, slot q):
                in1[p, 256:512] = hist_chunk(chunk_id[r, q])
                in0[p, 256:512] = per-element summed rec_w at the edge
                                  offsets inside that chunk (else 0)
  One scalar_tensor_tensor with accum_out produces 128 partial sums; a
  single tensor-engine matmul with the 0/1 matrix R[p,r]=(p//32==r)
  contracts them; ACT applies tanh(pre + bias); 4 results DMA out.

The inputs ride two DMAs on separate HWDGE queues into one SBUF tile;
block-strided access patterns present {W|Mw} and {h|histch} as single
operands.  A post-compile surgery pass hides the semaphore reset and one
all-engine barrier under the NEFF engine preamble and hoists the DMAs to
the entry block.

If an output row's edges ever reference more than 32 distinct chunks
(not observed; ~impossible for this distribution), the kernel falls back
to an indirect-DMA gather variant.
"""

import sys

for _p in ("/opt/trn_rl_repo", "/root/.axon_site/_ro/trn_rl_repo"):
    if _p not in sys.path:
        sys.path.insert(0, _p)

import numpy as np

import concourse.bacc as bacc
import concourse.bass as bass
import concourse.mybir as mybir
from concourse.bass_utils import run_bass_kernel_spmd
from concourse.tile import TileContext

N = 8192
N_IN = 64
N_OUT = 32
D = 4
N_CORES = 8
R_PER_CORE = N_OUT // N_CORES      # 4 output rows per core
KC = 128 // R_PER_CORE             # 32 partition groups per row
CHUNK = N // KC                    # 256 dense columns per partition
CH = 256                           # history page size (elements)
SLOTS = KC                         # history pages per output row
NSMALL = R_PER_CORE + 1            # rmat(4) + bias(1)
TCOLS = 4 * CH + NSMALL            # W | h | Mw | histch | small
CAP_E = 128                        # fallback gather capacity

_F32 = mybir.dt.float32
_I32 = mybir.dt.int32

_NC = {}


def _build_nc_fused():
    nc = bacc.Bacc(
        "TRN2", target_bir_lowering=False, debug=False, num_devices=N_CORES
    )
    biga = nc.dram_tensor("biga", [128, 2 * CH], _F32, kind="ExternalInput")
    bigb = nc.dram_tensor("bigb", [128, 2 * CH + NSMALL], _F32, kind="ExternalInput")
    out = nc.dram_tensor("out", [R_PER_CORE, 1], _F32, kind="ExternalOutput")

    with TileContext(nc) as tc:
        with (
            tc.tile_pool(name="sbuf", bufs=1) as pool,
            tc.tile_pool(name="psum", bufs=1, space="PSUM") as pp,
        ):
            t = pool.tile([128, TCOLS], _F32)
            nc.sync.dma_start(out=t[:, 0 : 2 * CH], in_=biga[:, :])
            nc.scalar.dma_start(out=t[:, 2 * CH : TCOLS], in_=bigb[:, :])

            r_t = t[:, 4 * CH : 4 * CH + R_PER_CORE]
            b_t = t[0:R_PER_CORE, 4 * CH + R_PER_CORE : 4 * CH + R_PER_CORE + 1]

            prod = pool.tile([128, 2 * CH], _F32)
            acc = pool.tile([128, 1], _F32)
            nc.vector.scalar_tensor_tensor(
                out=prod[:, :],
                in0=t[:, 0 : 2 * CH],
                scalar=1.0,
                in1=t[:, 2 * CH : 4 * CH],
                op0=mybir.AluOpType.mult,
                op1=mybir.AluOpType.mult,
                accum_out=acc[:, :],
            )

            ps = pp.tile([R_PER_CORE, 1], _F32)
            nc.tensor.matmul(
                out=ps[:, :], lhsT=r_t, rhs=acc[:, :], start=True, stop=True
            )

            o_t = pool.tile([R_PER_CORE, 1], _F32)
            nc.scalar.activation(
                o_t[:, :],
                ps[:, :],
                mybir.ActivationFunctionType.Tanh,
                bias=b_t,
                scale=1.0,
            )
            nc.sync.dma_start(out=out[:, :], in_=o_t[:, :], single_packet=True)

    nc.compile()
    _surgery(nc)
    return nc


def _build_nc_gather(cap=CAP_E):
    nc = bacc.Bacc(
        "TRN2", target_bir_lowering=False, debug=False, num_devices=N_CORES
    )
    nsm = 2 * R_PER_CORE + 2
    small = nc.dram_tensor("small", [128, nsm], _F32, kind="ExternalInput")
    big = nc.dram_tensor("big", [128, 2 * CHUNK], _F32, kind="ExternalInput")
    hist = nc.dram_tensor("hist", [N * D, 1], _F32, kind="ExternalInput")
    out = nc.dram_tensor("out", [R_PER_CORE, 1], _F32, kind="ExternalOutput")

    with TileContext(nc) as tc:
        with (
            tc.tile_pool(name="sbuf", bufs=1) as pool,
            tc.tile_pool(name="psum", bufs=1, space="PSUM") as pp,
        ):
            sm_t = pool.tile([128, nsm], _F32)
            nc.sync.dma_start(out=sm_t[:, :], in_=small[:, :], single_packet=True)
            r_t = sm_t[:, 0:R_PER_CORE]
            s_t = sm_t[0:cap, R_PER_CORE : 2 * R_PER_CORE]
            e_t = sm_t[0:cap, 2 * R_PER_CORE : 2 * R_PER_CORE + 1].bitcast(_I32)
            b_t = sm_t[0:R_PER_CORE, 2 * R_PER_CORE + 1 : 2 * R_PER_CORE + 2]

            g_t = pool.tile([cap, 1], _F32)
            nc.gpsimd.indirect_dma_start(
                out=g_t[:, :],
                out_offset=None,
                in_=hist[:, :],
                in_offset=bass.IndirectOffsetOnAxis(ap=e_t, axis=0),
            )

            w_t = pool.tile([128, 2 * CHUNK], _F32)
            nc.scalar.dma_start(out=w_t[:, :], in_=big[:, :])

            prod = pool.tile([128, CHUNK], _F32)
            part = pool.tile([128, 1], _F32)
            nc.vector.scalar_tensor_tensor(
                out=prod[:, :],
                in0=w_t[:, 0:CHUNK],
                scalar=1.0,
                in1=w_t[:, CHUNK : 2 * CHUNK],
                op0=mybir.AluOpType.mult,
                op1=mybir.AluOpType.mult,
                accum_out=part[:, :],
            )

            ps = pp.tile([R_PER_CORE, 1], _F32)
            nc.tensor.matmul(
                out=ps[:, :], lhsT=r_t, rhs=part[:, :], start=True, stop=False
            )
            nc.tensor.matmul(
                out=ps[:, :], lhsT=s_t, rhs=g_t[:, :], start=False, stop=True
            )

            o_t = pool.tile([R_PER_CORE, 1], _F32)
            nc.scalar.activation(
                o_t[:, :],
                ps[:, :],
                mybir.ActivationFunctionType.Tanh,
                bias=b_t,
                scale=1.0,
            )
            nc.sync.dma_start(out=out[:, :], in_=o_t[:, :], single_packet=True)

    nc.compile()
    _surgery(nc)
    return nc


def _surgery(nc):
    """Post-compile BIR surgery to shave fixed overhead:
    - drop the entry-block const memsets + all-engine barrier,
    - move the semaphore reset + range-clear + one all-engine barrier from
      the tail into the entry block (hidden under the NEFF preamble),
    - hoist the input DMAs into the entry block right after that barrier,
    - drop the second tail barrier; keep only the SP flush waits + drain,
    - merge the three basic blocks into one linear stream.
    """
    blocks = nc.m.functions[0].blocks
    if len(blocks) != 3:
        return
    main_bb, tile_bb, end_bb = blocks[0], blocks[1], blocks[2]

    hoist, rest = [], []
    for i in tile_bb.instructions:
        if isinstance(i, mybir.InstDMACopy) and getattr(i.ins[0], "memref", None) in (
            "small",
            "big",
            "hist",
            "biga",
            "bigb",
        ):
            if getattr(i.ins[0], "memref", None) == "hist":
                i.single_packet = True  # one-engine completion for the gather
            hoist.append(i)
        else:
            rest.append(i)

    keep_head = [i for i in main_bb.instructions if isinstance(i, mybir.InstCall)]
    main_il = main_bb.instructions
    tile_il = tile_bb.instructions

    end_il = end_bb.instructions
    keep_end, entry_sync = [], []
    seen_isa = False
    for i in end_il:
        if isinstance(i, mybir.InstISA):
            entry_sync.append(i)
            seen_isa = True
        elif isinstance(i, mybir.InstDrain):
            if (
                i.engine == mybir.EngineType.SP
                and i.has_wait()
                and "barrier_" not in i.concise()
            ):
                keep_end.append(i)  # SP flush drain
            elif seen_isa:
                pass  # second-barrier drains: drop
            else:
                entry_sync.append(i)
        elif isinstance(i, mybir.InstEventSemaphore):
            if i.engine == mybir.EngineType.SP and not i.name.startswith("barrier"):
                keep_end.append(i)  # global-clock DMA waits
            elif seen_isa:
                pass  # second barrier: drop
            else:
                entry_sync.append(i)
        else:
            keep_end.append(i)

    pool_reset = [
        i
        for i in entry_sync
        if isinstance(i, mybir.InstISA)
        or (isinstance(i, mybir.InstDrain) and "is_reset_sema=True" in i.concise())
    ]
    barrier_a = [i for i in entry_sync if i not in pool_reset]

    body = [i for i in rest if not isinstance(i, mybir.InstUnconditionalBranch)]
    # Input DMAs issue before the entry barrier: their semaphore increments
    # trail the Pool range-clear by construction (issue+transfer+receipt far
    # exceeds Pool's two-instruction path to the clear), and the barrier's
    # engine drains then overlap the transfers.
    main_il[:] = keep_head + pool_reset + hoist + barrier_a + body + keep_end
    tile_il[:] = []
    end_il[:] = []
    del nc.m.functions[0].blocks[1:]


def _get_nc(mode):
    if mode not in _NC:
        if mode == "fused":
            _NC[mode] = _build_nc_fused()
        else:
            _NC[mode] = _build_nc_gather(CAP_E)
    return _NC[mode]


def _prep_in_maps(obs, W, bias, current, history, rec_w, rec_src, rec_dst, rec_delay):
    obs = np.asarray(obs, np.float32)
    W = np.asarray(W, np.float32)
    bias = np.asarray(bias, np.float32)
    current = np.asarray(current, np.float32)
    history = np.ascontiguousarray(np.asarray(history, np.float32))
    rec_w = np.asarray(rec_w, np.float32)
    rec_src = np.asarray(rec_src).astype(np.int64)
    rec_dst = np.asarray(rec_dst).astype(np.int64)
    rec_delay = np.asarray(rec_delay).astype(np.int64)

    h = np.concatenate([obs, current[N_IN:]]).astype(np.float32)
    histf = history.reshape(N * D)
    hrep = np.tile(h.reshape(KC, CHUNK), (R_PER_CORE, 1))
    flat_all = rec_src * D + rec_delay

    # ---- fused (paged-history) layout ----
    in_maps = []
    fits = True
    for core in range(N_CORES):
        r0 = N - N_OUT + R_PER_CORE * core
        biga = np.zeros((128, 2 * CH), np.float32)
        biga[:, 0:CH] = (
            W[r0 : r0 + R_PER_CORE].reshape(R_PER_CORE, KC, CHUNK).reshape(128, CHUNK)
        )

        bigb = np.zeros((128, 2 * CH + NSMALL), np.float32)
        bigb[:, 0:CH] = hrep
        for r in range(R_PER_CORE):
            sel = rec_dst == r0 + r
            fl = flat_all[sel]
            ws = rec_w[sel]
            chunks = fl // CH
            offs = fl % CH
            uc = np.unique(chunks)
            if uc.shape[0] > SLOTS:
                fits = False
                break
            slot = {int(c): j for j, c in enumerate(uc)}
            for j, c in enumerate(uc):
                bigb[r * KC + j, CH : 2 * CH] = histf[c * CH : (c + 1) * CH]
            prows = np.array([r * KC + slot[int(c)] for c in chunks], np.int64)
            np.add.at(biga, (prows, CH + offs), ws)
        if not fits:
            break
        # rmat + bias in the trailing small columns
        bigb[np.arange(128), 2 * CH + np.arange(128) // KC] = 1.0
        bigb[0:R_PER_CORE, 2 * CH + R_PER_CORE] = bias[r0 : r0 + R_PER_CORE]
        in_maps.append({"biga": biga, "bigb": bigb})

    if fits:
        return in_maps, "fused"

    # ---- fallback: indirect-DMA gather layout ----
    histc = histf.reshape(N * D, 1)
    in_maps = []
    for core in range(N_CORES):
        r0 = N - N_OUT + R_PER_CORE * core
        big = np.empty((128, 2 * CHUNK), np.float32)
        big[:, 0:CHUNK] = (
            W[r0 : r0 + R_PER_CORE].reshape(R_PER_CORE, KC, CHUNK).reshape(128, CHUNK)
        )
        big[:, CHUNK:] = hrep

        sel = (rec_dst >= r0) & (rec_dst < r0 + R_PER_CORE)
        ew = rec_w[sel]
        flat = flat_all[sel]
        edloc = rec_dst[sel] - r0
        ne = int(flat.shape[0])
        if ne > CAP_E:
            key = flat * N_OUT + edloc
            uk, inv = np.unique(key, return_inverse=True)
            acc = np.zeros(uk.shape[0], np.float32)
            np.add.at(acc, inv, ew)
            flat = (uk // N_OUT).astype(np.int64)
            edloc = (uk % N_OUT).astype(np.int64)
            ew = acc
            ne = uk.shape[0]
            assert ne <= CAP_E, f"edge capacity exceeded: {ne}"

        nsm = 2 * R_PER_CORE + 2
        small = np.zeros((128, nsm), np.float32)
        small[np.arange(128), np.arange(128) // KC] = 1.0
        small[np.arange(ne), R_PER_CORE + edloc] = ew
        eidx = np.zeros((CAP_E,), np.int32)
        eidx[:ne] = flat.astype(np.int32)
        small[0:CAP_E, 2 * R_PER_CORE] = eidx.view(np.float32)
        small[0:R_PER_CORE, 2 * R_PER_CORE + 1] = bias[r0 : r0 + R_PER_CORE]
        in_maps.append({"small": small, "big": big, "hist": histc})
    return in_maps, "gather"


def _run(in_maps, mode="fused", trace=False, **kw):
    nc = _get_nc(mode)
    res = run_bass_kernel_spmd(
        nc, in_maps, core_ids=list(range(N_CORES)), trace=trace, **kw
    )
    outs = [res.results[c]["out"].reshape(R_PER_CORE) for c in range(N_CORES)]
    return np.concatenate(outs).astype(np.float32), res


def kernel(**inputs):
    in_maps, mode = _prep_in_maps(**inputs)
    out, _ = _run(in_maps, mode, trace=False)
    return out


# revision 34
# speedup vs baseline: 1.0649x; 1.0204x over previous
"""Trainium2 Bass kernel for nn_NeuronGraph_43336220017086.

Reference semantics:
    h_prev = concat(obs, current[N_IN:])            # [N]
    pre    = W @ h_prev + bias                      # [N]
    pre[rec_dst] += rec_w * history[rec_src, rec_delay]
    return tanh(pre)[-N_OUT:]                       # [N_OUT]

Only the last N_OUT=32 rows of `pre` reach the output, so the kernel
computes exactly those rows.  Work is sharded across 8 NeuronCores:
core c owns output rows [N-32+4c, N-32+4c+4).

Per core everything is folded into ONE fused DVE multiply-accumulate over
partition p = r*32 + q (r = local output row, q = 0..31):
  - dense part: in0[p, 0:256]   = W[r0+r, q*256:(q+1)*256]
                in1[p, 0:256]   = h_prev[q*256:(q+1)*256]
  - sparse part (recurrent edges, paged): each referenced 256-wide chunk
    of history.reshape(-1) is shipped once per (row, slot q):
                in1[p, 256:512] = hist_chunk(chunk_id[r, q])
                in0[p, 256:512] = per-element summed rec_w at the edge
                                  offsets inside that chunk (else 0)
  One scalar_tensor_tensor with accum_out produces 128 partial sums; a
  single tensor-engine matmul with the 0/1 matrix R[p,r]=(p//32==r)
  contracts them; ACT applies tanh(pre + bias); 4 results DMA out.

The inputs ride two DMAs on separate HWDGE queues into one SBUF tile;
block-strided access patterns present {W|Mw} and {h|histch} as single
operands.  A post-compile surgery pass hides the semaphore reset and one
all-engine barrier under the NEFF engine preamble and hoists the DMAs to
the entry block.

If an output row's edges ever reference more than 32 distinct chunks
(not observed; ~impossible for this distribution), the kernel falls back
to an indirect-DMA gather variant.
"""

import sys

for _p in ("/opt/trn_rl_repo", "/root/.axon_site/_ro/trn_rl_repo"):
    if _p not in sys.path:
        sys.path.insert(0, _p)

import numpy as np

import concourse.bacc as bacc
import concourse.bass as bass
import concourse.mybir as mybir
from concourse.bass_utils import run_bass_kernel_spmd
from concourse.tile import TileContext

N = 8192
N_IN = 64
N_OUT = 32
D = 4
N_CORES = 8
R_PER_CORE = N_OUT // N_CORES      # 4 output rows per core
KC = 128 // R_PER_CORE             # 32 partition groups per row
CHUNK = N // KC                    # 256 dense columns per partition
CH = 256                           # dense block width per partition
CHS = 64                           # history page size (elements)
B0 = CH + CHS                      # in0/in1 block width
SLOTS = KC                         # history pages per output row
NSMALL = R_PER_CORE + 1            # rmat(4) + bias(1)
TCOLS = 2 * B0 + NSMALL            # [W|Mw] | [h|histch] | small
CAP_E = 128                        # fallback gather capacity

_F32 = mybir.dt.float32
_I32 = mybir.dt.int32

_NC = {}


def _build_nc_fused():
    nc = bacc.Bacc(
        "TRN2", target_bir_lowering=False, debug=False, num_devices=N_CORES
    )
    biga = nc.dram_tensor("biga", [128, B0], _F32, kind="ExternalInput")
    bigb = nc.dram_tensor("bigb", [128, B0 + NSMALL], _F32, kind="ExternalInput")
    out = nc.dram_tensor("out", [R_PER_CORE, 1], _F32, kind="ExternalOutput")

    with TileContext(nc) as tc:
        with (
            tc.tile_pool(name="sbuf", bufs=1) as pool,
            tc.tile_pool(name="psum", bufs=1, space="PSUM") as pp,
        ):
            t = pool.tile([128, TCOLS], _F32)
            nc.sync.dma_start(out=t[:, 0:B0], in_=biga[:, :])
            nc.scalar.dma_start(out=t[:, B0:TCOLS], in_=bigb[:, :])

            r_t = t[:, 2 * B0 : 2 * B0 + R_PER_CORE]
            b_t = t[0:R_PER_CORE, 2 * B0 + R_PER_CORE : 2 * B0 + R_PER_CORE + 1]

            prod = pool.tile([128, B0], _F32)
            acc = pool.tile([128, 1], _F32)
            nc.vector.scalar_tensor_tensor(
                out=prod[:, :],
                in0=t[:, 0:B0],
                scalar=1.0,
                in1=t[:, B0 : 2 * B0],
                op0=mybir.AluOpType.mult,
                op1=mybir.AluOpType.mult,
                accum_out=acc[:, :],
            )

            ps = pp.tile([R_PER_CORE, 1], _F32)
            nc.tensor.matmul(
                out=ps[:, :], lhsT=r_t, rhs=acc[:, :], start=True, stop=True
            )

            o_t = pool.tile([R_PER_CORE, 1], _F32)
            nc.scalar.activation(
                o_t[:, :],
                ps[:, :],
                mybir.ActivationFunctionType.Tanh,
                bias=b_t,
                scale=1.0,
            )
            nc.sync.dma_start(out=out[:, :], in_=o_t[:, :], single_packet=True)

    nc.compile()
    _surgery(nc)
    return nc


def _build_nc_gather(cap=CAP_E):
    nc = bacc.Bacc(
        "TRN2", target_bir_lowering=False, debug=False, num_devices=N_CORES
    )
    nsm = 2 * R_PER_CORE + 2
    small = nc.dram_tensor("small", [128, nsm], _F32, kind="ExternalInput")
    big = nc.dram_tensor("big", [128, 2 * CHUNK], _F32, kind="ExternalInput")
    hist = nc.dram_tensor("hist", [N * D, 1], _F32, kind="ExternalInput")
    out = nc.dram_tensor("out", [R_PER_CORE, 1], _F32, kind="ExternalOutput")

    with TileContext(nc) as tc:
        with (
            tc.tile_pool(name="sbuf", bufs=1) as pool,
            tc.tile_pool(name="psum", bufs=1, space="PSUM") as pp,
        ):
            sm_t = pool.tile([128, nsm], _F32)
            nc.sync.dma_start(out=sm_t[:, :], in_=small[:, :], single_packet=True)
            r_t = sm_t[:, 0:R_PER_CORE]
            s_t = sm_t[0:cap, R_PER_CORE : 2 * R_PER_CORE]
            e_t = sm_t[0:cap, 2 * R_PER_CORE : 2 * R_PER_CORE + 1].bitcast(_I32)
            b_t = sm_t[0:R_PER_CORE, 2 * R_PER_CORE + 1 : 2 * R_PER_CORE + 2]

            g_t = pool.tile([cap, 1], _F32)
            nc.gpsimd.indirect_dma_start(
                out=g_t[:, :],
                out_offset=None,
                in_=hist[:, :],
                in_offset=bass.IndirectOffsetOnAxis(ap=e_t, axis=0),
            )

            w_t = pool.tile([128, 2 * CHUNK], _F32)
            nc.scalar.dma_start(out=w_t[:, :], in_=big[:, :])

            prod = pool.tile([128, CHUNK], _F32)
            part = pool.tile([128, 1], _F32)
            nc.vector.scalar_tensor_tensor(
                out=prod[:, :],
                in0=w_t[:, 0:CHUNK],
                scalar=1.0,
                in1=w_t[:, CHUNK : 2 * CHUNK],
                op0=mybir.AluOpType.mult,
                op1=mybir.AluOpType.mult,
                accum_out=part[:, :],
            )

            ps = pp.tile([R_PER_CORE, 1], _F32)
            nc.tensor.matmul(
                out=ps[:, :], lhsT=r_t, rhs=part[:, :], start=True, stop=False
            )
            nc.tensor.matmul(
                out=ps[:, :], lhsT=s_t, rhs=g_t[:, :], start=False, stop=True
            )

            o_t = pool.tile([R_PER_CORE, 1], _F32)
            nc.scalar.activation(
                o_t[:, :],
                ps[:, :],
                mybir.ActivationFunctionType.Tanh,
                bias=b_t,
                scale=1.0,
            )
            nc.sync.dma_start(out=out[:, :], in_=o_t[:, :], single_packet=True)

    nc.compile()
    _surgery(nc)
    return nc


def _surgery(nc):
    """Post-compile BIR surgery to shave fixed overhead:
    - drop the entry-block const memsets + all-engine barrier,
    - move the semaphore reset + range-clear + one all-engine barrier from
      the tail into the entry block (hidden under the NEFF preamble),
    - hoist the input DMAs into the entry block right after that barrier,
    - drop the second tail barrier; keep only the SP flush waits + drain,
    - merge the three basic blocks into one linear stream.
    """
    blocks = nc.m.functions[0].blocks
    if len(blocks) != 3:
        return
    main_bb, tile_bb, end_bb = blocks[0], blocks[1], blocks[2]

    hoist, rest = [], []
    for i in tile_bb.instructions:
        if isinstance(i, mybir.InstDMACopy) and getattr(i.ins[0], "memref", None) in (
            "small",
            "big",
            "hist",
            "biga",
            "bigb",
        ):
            if getattr(i.ins[0], "memref", None) == "hist":
                i.single_packet = True  # one-engine completion for the gather
            hoist.append(i)
        else:
            rest.append(i)

    keep_head = [i for i in main_bb.instructions if isinstance(i, mybir.InstCall)]
    main_il = main_bb.instructions
    tile_il = tile_bb.instructions

    end_il = end_bb.instructions
    keep_end, entry_sync = [], []
    seen_isa = False
    for i in end_il:
        if isinstance(i, mybir.InstISA):
            entry_sync.append(i)
            seen_isa = True
        elif isinstance(i, mybir.InstDrain):
            if (
                i.engine == mybir.EngineType.SP
                and i.has_wait()
                and "barrier_" not in i.concise()
            ):
                keep_end.append(i)  # SP flush drain
            elif seen_isa:
                pass  # second-barrier drains: drop
            else:
                entry_sync.append(i)
        elif isinstance(i, mybir.InstEventSemaphore):
            if i.engine == mybir.EngineType.SP and not i.name.startswith("barrier"):
                keep_end.append(i)  # global-clock DMA waits
            elif seen_isa:
                pass  # second barrier: drop
            else:
                entry_sync.append(i)
        else:
            keep_end.append(i)

    pool_reset = [
        i
        for i in entry_sync
        if isinstance(i, mybir.InstISA)
        or (isinstance(i, mybir.InstDrain) and "is_reset_sema=True" in i.concise())
    ]
    barrier_a = [i for i in entry_sync if i not in pool_reset]

    body = [i for i in rest if not isinstance(i, mybir.InstUnconditionalBranch)]
    # Input DMAs issue before the entry barrier: their semaphore increments
    # trail the Pool range-clear by construction (issue+transfer+receipt far
    # exceeds Pool's two-instruction path to the clear), and the barrier's
    # engine drains then overlap the transfers.
    main_il[:] = keep_head + pool_reset + hoist + barrier_a + body + keep_end
    tile_il[:] = []
    end_il[:] = []
    del nc.m.functions[0].blocks[1:]


def _get_nc(mode):
    if mode not in _NC:
        if mode == "fused":
            _NC[mode] = _build_nc_fused()
        else:
            _NC[mode] = _build_nc_gather(CAP_E)
    return _NC[mode]


def _prep_in_maps(obs, W, bias, current, history, rec_w, rec_src, rec_dst, rec_delay):
    obs = np.asarray(obs, np.float32)
    W = np.asarray(W, np.float32)
    bias = np.asarray(bias, np.float32)
    current = np.asarray(current, np.float32)
    history = np.ascontiguousarray(np.asarray(history, np.float32))
    rec_w = np.asarray(rec_w, np.float32)
    rec_src = np.asarray(rec_src).astype(np.int64)
    rec_dst = np.asarray(rec_dst).astype(np.int64)
    rec_delay = np.asarray(rec_delay).astype(np.int64)

    h = np.concatenate([obs, current[N_IN:]]).astype(np.float32)
    histf = history.reshape(N * D)
    hrep = np.tile(h.reshape(KC, CHUNK), (R_PER_CORE, 1))
    flat_all = rec_src * D + rec_delay

    # ---- fused (paged-history) layout ----
    in_maps = []
    fits = True
    for core in range(N_CORES):
        r0 = N - N_OUT + R_PER_CORE * core
        biga = np.zeros((128, B0), np.float32)
        biga[:, 0:CH] = (
            W[r0 : r0 + R_PER_CORE].reshape(R_PER_CORE, KC, CHUNK).reshape(128, CHUNK)
        )

        bigb = np.zeros((128, B0 + NSMALL), np.float32)
        bigb[:, 0:CH] = hrep
        for r in range(R_PER_CORE):
            sel = rec_dst == r0 + r
            fl = flat_all[sel]
            ws = rec_w[sel]
            chunks = fl // CHS
            offs = fl % CHS
            uc = np.unique(chunks)
            if uc.shape[0] > SLOTS:
                fits = False
                break
            slot = {int(c): j for j, c in enumerate(uc)}
            for j, c in enumerate(uc):
                bigb[r * KC + j, CH : CH + CHS] = histf[c * CHS : (c + 1) * CHS]
            prows = np.array([r * KC + slot[int(c)] for c in chunks], np.int64)
            np.add.at(biga, (prows, CH + offs), ws)
        if not fits:
            break
        # rmat + bias in the trailing small columns
        bigb[np.arange(128), B0 + np.arange(128) // KC] = 1.0
        bigb[0:R_PER_CORE, B0 + R_PER_CORE] = bias[r0 : r0 + R_PER_CORE]
        in_maps.append({"biga": biga, "bigb": bigb})

    if fits:
        return in_maps, "fused"

    # ---- fallback: indirect-DMA gather layout ----
    histc = histf.reshape(N * D, 1)
    in_maps = []
    for core in range(N_CORES):
        r0 = N - N_OUT + R_PER_CORE * core
        big = np.empty((128, 2 * CHUNK), np.float32)
        big[:, 0:CHUNK] = (
            W[r0 : r0 + R_PER_CORE].reshape(R_PER_CORE, KC, CHUNK).reshape(128, CHUNK)
        )
        big[:, CHUNK:] = hrep

        sel = (rec_dst >= r0) & (rec_dst < r0 + R_PER_CORE)
        ew = rec_w[sel]
        flat = flat_all[sel]
        edloc = rec_dst[sel] - r0
        ne = int(flat.shape[0])
        if ne > CAP_E:
            key = flat * N_OUT + edloc
            uk, inv = np.unique(key, return_inverse=True)
            acc = np.zeros(uk.shape[0], np.float32)
            np.add.at(acc, inv, ew)
            flat = (uk // N_OUT).astype(np.int64)
            edloc = (uk % N_OUT).astype(np.int64)
            ew = acc
            ne = uk.shape[0]
            assert ne <= CAP_E, f"edge capacity exceeded: {ne}"

        nsm = 2 * R_PER_CORE + 2
        small = np.zeros((128, nsm), np.float32)
        small[np.arange(128), np.arange(128) // KC] = 1.0
        small[np.arange(ne), R_PER_CORE + edloc] = ew
        eidx = np.zeros((CAP_E,), np.int32)
        eidx[:ne] = flat.astype(np.int32)
        small[0:CAP_E, 2 * R_PER_CORE] = eidx.view(np.float32)
        small[0:R_PER_CORE, 2 * R_PER_CORE + 1] = bias[r0 : r0 + R_PER_CORE]
        in_maps.append({"small": small, "big": big, "hist": histc})
    return in_maps, "gather"


def _run(in_maps, mode="fused", trace=False, **kw):
    nc = _get_nc(mode)
    res = run_bass_kernel_spmd(
        nc, in_maps, core_ids=list(range(N_CORES)), trace=trace, **kw
    )
    outs = [res.results[c]["out"].reshape(R_PER_CORE) for c in range(N_CORES)]
    return np.concatenate(outs).astype(np.float32), res


def kernel(**inputs):
    in_maps, mode = _prep_in_maps(**inputs)
    out, _ = _run(in_maps, mode, trace=False)
    return out


# revision 35
# speedup vs baseline: 1.0686x; 1.0034x over previous
"""Trainium2 Bass kernel for nn_NeuronGraph_43336220017086.

Reference semantics:
    h_prev = concat(obs, current[N_IN:])            # [N]
    pre    = W @ h_prev + bias                      # [N]
    pre[rec_dst] += rec_w * history[rec_src, rec_delay]
    return tanh(pre)[-N_OUT:]                       # [N_OUT]

Only the last N_OUT=32 rows of `pre` reach the output, so the kernel
computes exactly those rows.  Work is sharded across 8 NeuronCores:
core c owns output rows [N-32+4c, N-32+4c+4).

Per core everything is folded into ONE fused DVE multiply-accumulate over
partition p = r*32 + q (r = local output row, q = 0..31):
  - dense part: in0[p, 0:256]   = W[r0+r, q*256:(q+1)*256]
                in1[p, 0:256]   = h_prev[q*256:(q+1)*256]
  - sparse part (recurrent edges, paged): each referenced 256-wide chunk
    of history.reshape(-1) is shipped once per (row, slot q):
                in1[p, 256:512] = hist_chunk(chunk_id[r, q])
                in0[p, 256:512] = per-element summed rec_w at the edge
                                  offsets inside that chunk (else 0)
  One scalar_tensor_tensor with accum_out produces 128 partial sums; a
  single tensor-engine matmul with the 0/1 matrix R[p,r]=(p//32==r)
  contracts them; ACT applies tanh(pre + bias); 4 results DMA out.

The inputs ride two DMAs on separate HWDGE queues into one SBUF tile;
block-strided access patterns present {W|Mw} and {h|histch} as single
operands.  A post-compile surgery pass hides the semaphore reset and one
all-engine barrier under the NEFF engine preamble and hoists the DMAs to
the entry block.

If an output row's edges ever reference more than 32 distinct chunks
(not observed; ~impossible for this distribution), the kernel falls back
to an indirect-DMA gather variant.
"""

import sys

for _p in ("/opt/trn_rl_repo", "/root/.axon_site/_ro/trn_rl_repo"):
    if _p not in sys.path:
        sys.path.insert(0, _p)

import numpy as np

import concourse.bacc as bacc
import concourse.bass as bass
import concourse.mybir as mybir
from concourse.bass_utils import run_bass_kernel_spmd
from concourse.tile import TileContext

N = 8192
N_IN = 64
N_OUT = 32
D = 4
N_CORES = 8
R_PER_CORE = N_OUT // N_CORES      # 4 output rows per core
KC = 128 // R_PER_CORE             # 32 partition groups per row
CHUNK = N // KC                    # 256 dense columns per partition
CH = 256                           # dense block width per partition
CHS = 32                           # history page size (elements)
B0 = CH + CHS                      # in0/in1 block width
SLOTS = KC                         # history pages per output row
NSMALL = R_PER_CORE + 1            # rmat(4) + bias(1)
TCOLS = 2 * B0 + NSMALL            # [W|Mw] | [h|histch] | small
CAP_E = 128                        # fallback gather capacity

_F32 = mybir.dt.float32
_I32 = mybir.dt.int32

_NC = {}


def _build_nc_fused():
    nc = bacc.Bacc(
        "TRN2", target_bir_lowering=False, debug=False, num_devices=N_CORES
    )
    biga = nc.dram_tensor("biga", [128, B0], _F32, kind="ExternalInput")
    bigb = nc.dram_tensor("bigb", [128, B0 + NSMALL], _F32, kind="ExternalInput")
    out = nc.dram_tensor("out", [R_PER_CORE, 1], _F32, kind="ExternalOutput")

    with TileContext(nc) as tc:
        with (
            tc.tile_pool(name="sbuf", bufs=1) as pool,
            tc.tile_pool(name="psum", bufs=1, space="PSUM") as pp,
        ):
            t = pool.tile([128, TCOLS], _F32)
            nc.sync.dma_start(out=t[:, 0:B0], in_=biga[:, :])
            nc.scalar.dma_start(out=t[:, B0:TCOLS], in_=bigb[:, :])

            r_t = t[:, 2 * B0 : 2 * B0 + R_PER_CORE]
            b_t = t[0:R_PER_CORE, 2 * B0 + R_PER_CORE : 2 * B0 + R_PER_CORE + 1]

            prod = pool.tile([128, B0], _F32)
            acc = pool.tile([128, 1], _F32)
            nc.vector.scalar_tensor_tensor(
                out=prod[:, :],
                in0=t[:, 0:B0],
                scalar=1.0,
                in1=t[:, B0 : 2 * B0],
                op0=mybir.AluOpType.mult,
                op1=mybir.AluOpType.mult,
                accum_out=acc[:, :],
            )

            ps = pp.tile([R_PER_CORE, 1], _F32)
            nc.tensor.matmul(
                out=ps[:, :], lhsT=r_t, rhs=acc[:, :], start=True, stop=True
            )

            o_t = pool.tile([R_PER_CORE, 1], _F32)
            nc.scalar.activation(
                o_t[:, :],
                ps[:, :],
                mybir.ActivationFunctionType.Tanh,
                bias=b_t,
                scale=1.0,
            )
            nc.sync.dma_start(out=out[:, :], in_=o_t[:, :], single_packet=True)

    nc.compile()
    _surgery(nc)
    return nc


def _build_nc_gather(cap=CAP_E):
    nc = bacc.Bacc(
        "TRN2", target_bir_lowering=False, debug=False, num_devices=N_CORES
    )
    nsm = 2 * R_PER_CORE + 2
    small = nc.dram_tensor("small", [128, nsm], _F32, kind="ExternalInput")
    big = nc.dram_tensor("big", [128, 2 * CHUNK], _F32, kind="ExternalInput")
    hist = nc.dram_tensor("hist", [N * D, 1], _F32, kind="ExternalInput")
    out = nc.dram_tensor("out", [R_PER_CORE, 1], _F32, kind="ExternalOutput")

    with TileContext(nc) as tc:
        with (
            tc.tile_pool(name="sbuf", bufs=1) as pool,
            tc.tile_pool(name="psum", bufs=1, space="PSUM") as pp,
        ):
            sm_t = pool.tile([128, nsm], _F32)
            nc.sync.dma_start(out=sm_t[:, :], in_=small[:, :], single_packet=True)
            r_t = sm_t[:, 0:R_PER_CORE]
            s_t = sm_t[0:cap, R_PER_CORE : 2 * R_PER_CORE]
            e_t = sm_t[0:cap, 2 * R_PER_CORE : 2 * R_PER_CORE + 1].bitcast(_I32)
            b_t = sm_t[0:R_PER_CORE, 2 * R_PER_CORE + 1 : 2 * R_PER_CORE + 2]

            g_t = pool.tile([cap, 1], _F32)
            nc.gpsimd.indirect_dma_start(
                out=g_t[:, :],
                out_offset=None,
                in_=hist[:, :],
                in_offset=bass.IndirectOffsetOnAxis(ap=e_t, axis=0),
            )

            w_t = pool.tile([128, 2 * CHUNK], _F32)
            nc.scalar.dma_start(out=w_t[:, :], in_=big[:, :])

            prod = pool.tile([128, CHUNK], _F32)
            part = pool.tile([128, 1], _F32)
            nc.vector.scalar_tensor_tensor(
                out=prod[:, :],
                in0=w_t[:, 0:CHUNK],
                scalar=1.0,
                in1=w_t[:, CHUNK : 2 * CHUNK],
                op0=mybir.AluOpType.mult,
                op1=mybir.AluOpType.mult,
                accum_out=part[:, :],
            )

            ps = pp.tile([R_PER_CORE, 1], _F32)
            nc.tensor.matmul(
                out=ps[:, :], lhsT=r_t, rhs=part[:, :], start=True, stop=False
            )
            nc.tensor.matmul(
                out=ps[:, :], lhsT=s_t, rhs=g_t[:, :], start=False, stop=True
            )

            o_t = pool.tile([R_PER_CORE, 1], _F32)
            nc.scalar.activation(
                o_t[:, :],
                ps[:, :],
                mybir.ActivationFunctionType.Tanh,
                bias=b_t,
                scale=1.0,
            )
            nc.sync.dma_start(out=out[:, :], in_=o_t[:, :], single_packet=True)

    nc.compile()
    _surgery(nc)
    return nc


def _surgery(nc):
    """Post-compile BIR surgery to shave fixed overhead:
    - drop the entry-block const memsets + all-engine barrier,
    - move the semaphore reset + range-clear + one all-engine barrier from
      the tail into the entry block (hidden under the NEFF preamble),
    - hoist the input DMAs into the entry block right after that barrier,
    - drop the second tail barrier; keep only the SP flush waits + drain,
    - merge the three basic blocks into one linear stream.
    """
    blocks = nc.m.functions[0].blocks
    if len(blocks) != 3:
        return
    main_bb, tile_bb, end_bb = blocks[0], blocks[1], blocks[2]

    hoist, rest = [], []
    for i in tile_bb.instructions:
        if isinstance(i, mybir.InstDMACopy) and getattr(i.ins[0], "memref", None) in (
            "small",
            "big",
            "hist",
            "biga",
            "bigb",
        ):
            if getattr(i.ins[0], "memref", None) == "hist":
                i.single_packet = True  # one-engine completion for the gather
            hoist.append(i)
        else:
            rest.append(i)

    keep_head = [i for i in main_bb.instructions if isinstance(i, mybir.InstCall)]
    main_il = main_bb.instructions
    tile_il = tile_bb.instructions

    end_il = end_bb.instructions
    keep_end, entry_sync = [], []
    seen_isa = False
    for i in end_il:
        if isinstance(i, mybir.InstISA):
            entry_sync.append(i)
            seen_isa = True
        elif isinstance(i, mybir.InstDrain):
            if (
                i.engine == mybir.EngineType.SP
                and i.has_wait()
                and "barrier_" not in i.concise()
            ):
                keep_end.append(i)  # SP flush drain
            elif seen_isa:
                pass  # second-barrier drains: drop
            else:
                entry_sync.append(i)
        elif isinstance(i, mybir.InstEventSemaphore):
            if i.engine == mybir.EngineType.SP and not i.name.startswith("barrier"):
                keep_end.append(i)  # global-clock DMA waits
            elif seen_isa:
                pass  # second barrier: drop
            else:
                entry_sync.append(i)
        else:
            keep_end.append(i)

    pool_reset = [
        i
        for i in entry_sync
        if isinstance(i, mybir.InstISA)
        or (isinstance(i, mybir.InstDrain) and "is_reset_sema=True" in i.concise())
    ]
    barrier_a = [i for i in entry_sync if i not in pool_reset]

    body = [i for i in rest if not isinstance(i, mybir.InstUnconditionalBranch)]
    # Input DMAs issue before the entry barrier: their semaphore increments
    # trail the Pool range-clear by construction (issue+transfer+receipt far
    # exceeds Pool's two-instruction path to the clear), and the barrier's
    # engine drains then overlap the transfers.
    main_il[:] = keep_head + pool_reset + hoist + barrier_a + body + keep_end
    tile_il[:] = []
    end_il[:] = []
    del nc.m.functions[0].blocks[1:]


def _get_nc(mode):
    if mode not in _NC:
        if mode == "fused":
            _NC[mode] = _build_nc_fused()
        else:
            _NC[mode] = _build_nc_gather(CAP_E)
    return _NC[mode]


def _prep_in_maps(obs, W, bias, current, history, rec_w, rec_src, rec_dst, rec_delay):
    obs = np.asarray(obs, np.float32)
    W = np.asarray(W, np.float32)
    bias = np.asarray(bias, np.float32)
    current = np.asarray(current, np.float32)
    history = np.ascontiguousarray(np.asarray(history, np.float32))
    rec_w = np.asarray(rec_w, np.float32)
    rec_src = np.asarray(rec_src).astype(np.int64)
    rec_dst = np.asarray(rec_dst).astype(np.int64)
    rec_delay = np.asarray(rec_delay).astype(np.int64)

    h = np.concatenate([obs, current[N_IN:]]).astype(np.float32)
    histf = history.reshape(N * D)
    hrep = np.tile(h.reshape(KC, CHUNK), (R_PER_CORE, 1))
    flat_all = rec_src * D + rec_delay

    # ---- fused (paged-history) layout ----
    in_maps = []
    fits = True
    for core in range(N_CORES):
        r0 = N - N_OUT + R_PER_CORE * core
        biga = np.zeros((128, B0), np.float32)
        biga[:, 0:CH] = (
            W[r0 : r0 + R_PER_CORE].reshape(R_PER_CORE, KC, CHUNK).reshape(128, CHUNK)
        )

        bigb = np.zeros((128, B0 + NSMALL), np.float32)
        bigb[:, 0:CH] = hrep
        for r in range(R_PER_CORE):
            sel = rec_dst == r0 + r
            fl = flat_all[sel]
            ws = rec_w[sel]
            chunks = fl // CHS
            offs = fl % CHS
            uc = np.unique(chunks)
            if uc.shape[0] > SLOTS:
                fits = False
                break
            slot = {int(c): j for j, c in enumerate(uc)}
            for j, c in enumerate(uc):
                bigb[r * KC + j, CH : CH + CHS] = histf[c * CHS : (c + 1) * CHS]
            prows = np.array([r * KC + slot[int(c)] for c in chunks], np.int64)
            np.add.at(biga, (prows, CH + offs), ws)
        if not fits:
            break
        # rmat + bias in the trailing small columns
        bigb[np.arange(128), B0 + np.arange(128) // KC] = 1.0
        bigb[0:R_PER_CORE, B0 + R_PER_CORE] = bias[r0 : r0 + R_PER_CORE]
        in_maps.append({"biga": biga, "bigb": bigb})

    if fits:
        return in_maps, "fused"

    # ---- fallback: indirect-DMA gather layout ----
    histc = histf.reshape(N * D, 1)
    in_maps = []
    for core in range(N_CORES):
        r0 = N - N_OUT + R_PER_CORE * core
        big = np.empty((128, 2 * CHUNK), np.float32)
        big[:, 0:CHUNK] = (
            W[r0 : r0 + R_PER_CORE].reshape(R_PER_CORE, KC, CHUNK).reshape(128, CHUNK)
        )
        big[:, CHUNK:] = hrep

        sel = (rec_dst >= r0) & (rec_dst < r0 + R_PER_CORE)
        ew = rec_w[sel]
        flat = flat_all[sel]
        edloc = rec_dst[sel] - r0
        ne = int(flat.shape[0])
        if ne > CAP_E:
            key = flat * N_OUT + edloc
            uk, inv = np.unique(key, return_inverse=True)
            acc = np.zeros(uk.shape[0], np.float32)
            np.add.at(acc, inv, ew)
            flat = (uk // N_OUT).astype(np.int64)
            edloc = (uk % N_OUT).astype(np.int64)
            ew = acc
            ne = uk.shape[0]
            assert ne <= CAP_E, f"edge capacity exceeded: {ne}"

        nsm = 2 * R_PER_CORE + 2
        small = np.zeros((128, nsm), np.float32)
        small[np.arange(128), np.arange(128) // KC] = 1.0
        small[np.arange(ne), R_PER_CORE + edloc] = ew
        eidx = np.zeros((CAP_E,), np.int32)
        eidx[:ne] = flat.astype(np.int32)
        small[0:CAP_E, 2 * R_PER_CORE] = eidx.view(np.float32)
        small[0:R_PER_CORE, 2 * R_PER_CORE + 1] = bias[r0 : r0 + R_PER_CORE]
        in_maps.append({"small": small, "big": big, "hist": histc})
    return in_maps, "gather"


def _run(in_maps, mode="fused", trace=False, **kw):
    nc = _get_nc(mode)
    res = run_bass_kernel_spmd(
        nc, in_maps, core_ids=list(range(N_CORES)), trace=trace, **kw
    )
    outs = [res.results[c]["out"].reshape(R_PER_CORE) for c in range(N_CORES)]
    return np.concatenate(outs).astype(np.float32), res


def kernel(**inputs):
    in_maps, mode = _prep_in_maps(**inputs)
    out, _ = _run(in_maps, mode, trace=False)
    return out
